# revision 10
# baseline (speedup 1.0000x reference)
"""Trainium2 Bass kernel for AdvancedLearnableEntropyPooling2D.

Math (per 2x2 window, per channel):
    s  = sum_i exp(x_i)                 (i over the 4 window elements)
    T  = sum_i x_i * exp(x_i)
    H  = log(s) - T/s                   (= softmax entropy; the reference's
                                         +1e-6 inside log contributes ~4e-6)
    out = H * w[c] + b[c]

Sharding: pure data parallel over batch (32 / 8 cores = 4 per core).

Layout per core: process "superunits" of 16 input rows (8 window rows) of one
batch image. SBUF partitions carry 128 row-pixels (contiguous 512B HBM chunks
per partition); free dim carries channels. Window sums are done on TensorE
with a 0/1 pixel-pair matrix as stationary weights (float32r, free-dim 256
-> full rate), accumulating the vertical row pair into PSUM. ScalarE: exp/log.
VectorE/GPSIMD: the e*x product and the small post ops.
"""

import numpy as np

import concourse.bass as bass
import concourse.bacc as bacc
import concourse.mybir as mybir
from concourse.bass_utils import run_bass_kernel_spmd
from concourse.tile import TileContext

AF = mybir.ActivationFunctionType
ALU = mybir.AluOpType
DT = mybir.dt

B, H, W, C = 32, 256, 256, 128
N_CORES = 8
BP = B // N_CORES            # 4 batch images per core
HO, WO = H // 2, W // 2      # 128, 128
SU_ROWS = 16                 # input rows per superunit
WRS = SU_ROWS // 2           # 8 window rows per superunit
N_SU = BP * (H // SU_ROWS)   # 64 superunits per core
NPOST = WRS * C              # 1024 post-stage elems per partition

TRACE = False
LAST_EXEC_NS = None
_NC_CACHE = None


def _build_nc():
    nc = bacc.Bacc(None, target_bir_lowering=False)
    x = nc.dram_tensor("x", [BP, H, W, C], DT.float32, kind="ExternalInput")
    m_pair = nc.dram_tensor("m_pair", [128, 256], DT.float32, kind="ExternalInput")
    w_bc = nc.dram_tensor("w_bc", [128, NPOST], DT.float32, kind="ExternalInput")
    b_bc = nc.dram_tensor("b_bc", [128, NPOST], DT.float32, kind="ExternalInput")
    out = nc.dram_tensor("out", [BP, HO, WO, C], DT.float32, kind="ExternalOutput")

    with TileContext(nc) as tc:
        with (
            tc.tile_pool(name="const", bufs=1) as cp,
            tc.tile_pool(name="xin", bufs=3) as xp,
            tc.tile_pool(name="eex", bufs=2) as ep,
            tc.tile_pool(name="post", bufs=2) as pp,
            tc.tile_pool(name="ob", bufs=3) as op_,
            tc.tile_pool(name="st", bufs=2, space="PSUM") as sp,
        ):
            m_t = cp.tile([128, 256], DT.float32)
            nc.sync.dma_start(m_t[:], m_pair[:])
            m_tr = cp.tile([128, 256], DT.float32r)
            nc.vector.tensor_copy(m_tr[:], m_t[:])
            w_t = cp.tile([128, NPOST], DT.float32)
            nc.sync.dma_start(w_t[:], w_bc[:])
            b_t = cp.tile([128, NPOST], DT.float32)
            nc.sync.dma_start(b_t[:], b_bc[:])

            for su in range(N_SU):
                b_i, hb = divmod(su, H // SU_ROWS)
                h0 = hb * SU_ROWS

                X = xp.tile([128, SU_ROWS * 2 * C], DT.float32, name=f"X{su}", tag="X")
                xv = X.rearrange("p (r t c) -> p r t c", r=SU_ROWS, t=2, c=C)
                src = x[b_i, h0 : h0 + SU_ROWS].rearrange(
                    "r (t p) c -> p r t c", t=2, p=128
                )
                nc.sync.dma_start(xv, src)

                EEX = ep.tile(
                    [128, SU_ROWS * 2 * 2 * C], DT.float32r, name=f"EEX{su}", tag="EEX"
                )
                ev = EEX.rearrange(
                    "p (r t y c) -> p r t y c", r=SU_ROWS, t=2, y=2, c=C
                )
                nc.scalar.activation(ev[:, :, :, 0, :], xv, AF.Exp)
                nc.vector.tensor_tensor(
                    ev[:, :, :, 1, :], ev[:, :, :, 0, :], xv, ALU.mult
                )
                ev2 = EEX.rearrange(
                    "p (r t yc) -> p r t yc", r=SU_ROWS, t=2, yc=2 * C
                )

                ST = sp.tile([128, WRS * 2 * C], DT.float32, name=f"ST{su}", tag="ST")
                stv = ST.rearrange("p (w y c) -> p w y c", w=WRS, y=2, c=C)
                for wr in range(WRS):
                    for t in range(2):
                        for r in range(2):
                            nc.tensor.matmul(
                                ST[:, wr * 2 * C : (wr + 1) * 2 * C],
                                lhsT=m_tr[:, 128 * t : 128 * (t + 1)],
                                rhs=ev2[:, 2 * wr + r, t, :],
                                start=(t == 0 and r == 0),
                                stop=(t == 1 and r == 1),
                            )

                S = stv[:, :, 0, :]  # [128, 8, 128] strided PSUM view
                T_ = stv[:, :, 1, :]

                logs = pp.tile([128, NPOST], DT.float32, name=f"lg{su}", tag="lg")
                lgv = logs.rearrange("p (w c) -> p w c", w=WRS, c=C)
                nc.scalar.activation(lgv, S, AF.Ln)

                rcp = pp.tile([128, NPOST], DT.float32, name=f"rc{su}", tag="rc")
                rcv = rcp.rearrange("p (w c) -> p w c", w=WRS, c=C)
                nc.vector.reciprocal(rcv, S)

                mean = pp.tile([128, NPOST], DT.float32, name=f"mn{su}", tag="mn")
                mnv = mean.rearrange("p (w c) -> p w c", w=WRS, c=C)
                nc.vector.tensor_tensor(mnv, T_, rcv, ALU.mult)

                z = pp.tile([128, NPOST], DT.float32, name=f"z{su}", tag="z")
                nc.gpsimd.tensor_tensor(z[:], logs[:], mean[:], ALU.subtract)

                zw = pp.tile([128, NPOST], DT.float32, name=f"zw{su}", tag="zw")
                nc.gpsimd.tensor_tensor(zw[:], z[:], w_t[:], ALU.mult)

                o = op_.tile([128, NPOST], DT.float32, name=f"o{su}", tag="o")
                nc.vector.tensor_tensor(o[:], zw[:], b_t[:], ALU.add)

                dst = out[b_i, hb * WRS : (hb + 1) * WRS].rearrange("h w c -> w h c")
                nc.sync.dma_start(dst, o.rearrange("p (h c) -> p h c", h=WRS, c=C))
    nc.finalize()
    return nc


def _host_consts(entropy_weights, pooling_bias):
    # m[:, 0:128] routes t=0 pixel pairs to PSUM rows 0..63;
    # m[:, 128:256] routes t=1 pixel pairs to PSUM rows 64..127.
    m = np.zeros((128, 256), dtype=np.float32)
    for wo in range(64):
        m[2 * wo, wo] = 1.0
        m[2 * wo + 1, wo] = 1.0
        m[2 * wo, 128 + 64 + wo] = 1.0
        m[2 * wo + 1, 128 + 64 + wo] = 1.0
    w_bc = np.ascontiguousarray(
        np.broadcast_to(
            entropy_weights.astype(np.float32), (128, WRS, C)
        ).reshape(128, NPOST)
    )
    b_bc = np.ascontiguousarray(
        np.broadcast_to(
            pooling_bias.astype(np.float32), (128, WRS, C)
        ).reshape(128, NPOST)
    )
    return m, w_bc, b_bc


def kernel(x, entropy_weights, pooling_bias):
    global _NC_CACHE, LAST_EXEC_NS
    x = np.ascontiguousarray(np.asarray(x, dtype=np.float32))
    m, w_bc, b_bc = _host_consts(
        np.asarray(entropy_weights), np.asarray(pooling_bias)
    )
    if _NC_CACHE is None:
        _NC_CACHE = _build_nc()
    nc = _NC_CACHE

    core_ids = list(range(N_CORES))
    in_maps = [
        {
            "x": np.ascontiguousarray(x[i * BP : (i + 1) * BP]),
            "m_pair": m,
            "w_bc": w_bc,
            "b_bc": b_bc,
        }
        for i in core_ids
    ]
    res = run_bass_kernel_spmd(nc, in_maps, core_ids, trace=TRACE)
    LAST_EXEC_NS = res.exec_time_ns
    return np.concatenate([res.results[i]["out"] for i in range(N_CORES)], axis=0)
